# revision 7
# baseline (speedup 1.0000x reference)
"""Trainium2 Bass kernel for nn_CIN: 3-layer compressed-interaction network.

Reference (per layer l, kernel k_l [O,H,M]):
    x_{l+1}[b,o,d] = sum_{h,m} x_l[b,h,d] * x0[b,m,d] * k_l[o,h,m]
    out = concat_l(sum_d x_{l+1}[b,o,d])          # (B, 3*128)

Sharding: pure data-parallel over B across 8 cores (512 batch each).

v4 design (fp16 end-to-end; rationale from cost-model study):
  * matmul cost = out-cols x 0.42ns, independent of contraction depth
    => deep PSUM accumulation is free, narrow outputs are cheap.
  * DVE/ACT elementwise ops on PSUM run at 1x (~0.55-0.66us per
    [128,512]); all-2-byte SBUF DVE ops can run 2-4x.

  L0: u-SYM route. u[(m<=m'),bd] = x0m[m]*x0m[m'] (820 rows, host,
  fp16) with k0s[(m,m'),o] = k0[o,m,m']+k0[o,m',m] folded: 7 row-blocks
  instead of 13 -> halves L0 PE time AND u DMA vs v3.
  y[o,bd-512] = 7 accumulating matmuls per window; ACT-drain to x1b.

  L1: per chunk, 10 matmuls pt_g[bd',4m*128o] in PSUM (lhsT = x1b
  chunk, shared). The m-contraction x2[bd,o] = sum_m pt[bd,(m,o)] *
  x0t[bd,m] is a mandatory 42M-elem PSUM drain pass; it is split
  across three engines by a tunable table:
    'D': DVE  tensor_tensor drain fused with the x0 broadcast-scale
    'A': ACT  copy drain (fp16), then DVE applies the scale (2x, SBUF)
    'G': GPSIMD tensor_tensor drain fused with the scale
  then a 6-op DVE fold-tree (all fp16 SBUF) sums the 40 m-slices.
  No diag DMA, no diag matmuls, no v_act bulk copies (v3 killed).

  L2: Gram-trick as v3: per chunk one indicator matmul
  pw[o,(m',e)] = v[:,0:128].T @ (x0t*E) gives G2 + out2; tail runs
  per 128-b block (40 accumulating [*,128] matmuls) for overlap.
"""

import numpy as np
from contextlib import ExitStack

import concourse.bass as bass
import concourse.tile as tile
import concourse.mybir as mybir

F32 = mybir.dt.float32
F16 = mybir.dt.float16
ALU = mybir.AluOpType

B, M, D, O = 4096, 40, 16, 128
N_CORES = 8
BC = B // N_CORES          # 512 batch rows per core
M1 = M + 1                 # x0t carries a trailing ones-column
NPAIR = M * (M + 1) // 2   # 820 symmetric (m<=m') pairs for L0
RB = 7                     # L0 row-blocks (896 rows incl pad)
WIN = 4                    # chunks per L0 window (512 cols)
NG = 10                    # pt groups per chunk (4 m's each)

# per-(c%2) engine for each of the 5 drain PAIRS (2 groups = 2 PSUM
# banks each).  GPSIMD cannot read PSUM, so drains are DVE or ACT only:
# D = DVE tensor_tensor drain fused with the x0 scale
# A = ACT copy drain, DVE applies the scale on SBUF (2x)
# B = ACT copy drain, GPSIMD applies the scale on SBUF
DRAIN_TABLE = [
    "DABBA",
    "DDABB",
]

_ns_ctr = [0]


def _split_excess_waits(nc, max_waits=1):
    """walrus in this env rejects >1 sync-wait on one instruction: move
    excess waits onto same-engine NoOps inserted before."""
    for f in nc.m.functions:
        for bb in f.blocks:
            new_list = []
            for inst in bb.instructions:
                si = inst.sync_info
                waits = list(si.on_wait) if si and si.on_wait else []
                if len(waits) > max_waits:
                    excess = waits[:-max_waits]
                    keep = waits[-max_waits:]
                    for i in range(0, len(excess), max_waits):
                        chunk = excess[i:i + max_waits]
                        _ns_ctr[0] += 1
                        nop = mybir.InstNoOp(
                            name=f"waitsplit-{_ns_ctr[0]}", ins=[], outs=[],
                            engine=inst.engine,
                            sync_info=mybir.SyncInfo(on_wait=chunk,
                                                     on_update=[]),
                        )
                        nc.register_instruction(nop)
                        new_list.append(nop)
                    si.on_wait = keep
                    inst.sync_info = si
                new_list.append(inst)
            bb.instructions[:] = new_list


def build(n_chunks):
    bd = n_chunks * 128
    bcl = bd // D              # local batch count
    nb = (bcl + 127) // 128    # output b-tiles
    nwin = n_chunks // WIN
    nc = bass.Bass("TRN2", target_bir_lowering=False, debug=False,
                   num_devices=1)

    u_d = nc.dram_tensor("u", [RB * 128, bd], F16, kind="ExternalInput")
    k0s_d = nc.dram_tensor("k0s", [RB * 128, O], F16, kind="ExternalInput")
    k1p_d = nc.dram_tensor("k1p", [O, M * O], F16, kind="ExternalInput")
    k2p_d = nc.dram_tensor("k2p", [O, M * O], F16, kind="ExternalInput")
    x0t_d = nc.dram_tensor("x0t", [bd, M1], F16, kind="ExternalInput")
    x0e_d = nc.dram_tensor("x0e", [128, n_chunks * M1 * 8], F16,
                           kind="ExternalInput")
    iden_d = nc.dram_tensor("iden", [128, 128], F32, kind="ExternalInput")
    out_d = nc.dram_tensor("out", [bcl, 3 * O], F32, kind="ExternalOutput")

    with tile.TileContext(nc) as tc:
        with ExitStack() as perm:
            pp = perm.enter_context(tc.tile_pool(name="perm", bufs=1))
            k0s_sb = pp.tile([128, RB * O], F16, name="k0s_sb")
            nc.sync.dma_start(
                k0s_sb[:].rearrange("p (j o) -> p j o", o=O),
                k0s_d.ap().rearrange("(j p) o -> p j o", p=128))
            k1p_sb = pp.tile([O, M * O], F16, name="k1p_sb")
            nc.sync.dma_start(k1p_sb[:], k1p_d.ap())
            k2p_sb = pp.tile([O, M * O], F16, name="k2p_sb")
            nc.sync.dma_start(k2p_sb[:], k2p_d.ap())
            x0t_sb = pp.tile([128, n_chunks * M1], F16, name="x0t_sb")
            nc.sync.dma_start(
                x0t_sb[:].rearrange("p (c m) -> p c m", m=M1),
                x0t_d.ap().rearrange("(c p) m -> p c m", p=128))
            iden_sb = pp.tile([128, 128], F32, name="iden_sb")
            nc.sync.dma_start(iden_sb[:], iden_d.ap())

            x1b = pp.tile([128, bd], F16, name="x1b")
            w_sb = pp.tile([128, M1 * bcl], F16, name="w_sb")
            o1_st = pp.tile([128, bcl], F32, name="o1_st")
            o2_st = pp.tile([128, bcl], F32, name="o2_st")
            o3_st = pp.tile([128, bcl], F32, name="o3_st")

            w_4d = w_sb[:].rearrange("p (m b) -> p m b", b=bcl)

            with ExitStack() as mainst:
                u_pool = mainst.enter_context(
                    tc.tile_pool(name="upool", bufs=2))
                v_pool = mainst.enter_context(
                    tc.tile_pool(name="vpool", bufs=2))
                xe_pool = mainst.enter_context(
                    tc.tile_pool(name="xepool", bufs=2))
                y_pool = mainst.enter_context(
                    tc.tile_pool(name="ypool", bufs=2, space="PSUM"))
                pt_pool = mainst.enter_context(
                    tc.tile_pool(name="ptpool", bufs=2, space="PSUM"))
                pw_pool = mainst.enter_context(
                    tc.tile_pool(name="pwpool", bufs=1, space="PSUM"))
                po3_pool = mainst.enter_context(
                    tc.tile_pool(name="po3p", bufs=1, space="PSUM"))

                for w in range(nwin):
                    ws = slice(w * 512, (w + 1) * 512)
                    # ---- L0: 7 accumulating matmuls over host-built u-sym
                    u_sb = u_pool.tile([128, RB * 512], F16, name="u_sb",
                                       tag="u")
                    nc.sync.dma_start(
                        u_sb[:].rearrange("p (j c) -> p j c", c=512),
                        u_d.ap().rearrange("(j p) c -> p j c", p=128)
                        [:, :, ws])
                    xe = xe_pool.tile([128, WIN * M1 * 8], F16, name="xe",
                                      tag="xe")
                    nc.sync.dma_start(
                        xe[:], x0e_d.ap()[:, w * WIN * M1 * 8:
                                          (w + 1) * WIN * M1 * 8])
                    y = y_pool.tile([128, 512], F32, name="y", tag="y")
                    for j in range(RB):
                        nc.tensor.matmul(
                            y[:], k0s_sb[:, j * O:(j + 1) * O],
                            u_sb[:, j * 512:(j + 1) * 512],
                            start=(j == 0), stop=(j == RB - 1))
                    nc.scalar.copy(x1b[:, ws], y[:])

                    if w % 4 == 3:
                        q = w // 4
                        nc.vector.tensor_reduce(
                            o1_st[:, q * 128:(q + 1) * 128],
                            x1b[:, q * 2048:(q + 1) * 2048]
                            .rearrange("p (b d) -> p b d", d=D),
                            mybir.AxisListType.X, ALU.add)

                    for c in range(w * WIN, (w + 1) * WIN):
                        cs = slice(c * 128, (c + 1) * 128)
                        tbl = DRAIN_TABLE[c % len(DRAIN_TABLE)]

                        def x0bc(pr):
                            return (x0t_sb[:, c * M1 + 8 * pr:
                                           c * M1 + 8 * pr + 8]
                                    .unsqueeze(2).broadcast_to([128, 8, 128]))

                        # ---- L1: 5 matmul pair-groups + scaled drain
                        v = v_pool.tile([128, NG * 512], F16, name="v",
                                        tag="v")
                        v_3d = v[:].rearrange("p (m o) -> p m o", o=O)
                        for pr in range(NG // 2):
                            pt = pt_pool.tile([128, 1024], F32, name="pt",
                                              tag="pt")
                            ps = slice(pr * 1024, (pr + 1) * 1024)
                            for h in range(2):
                                nc.tensor.matmul(
                                    pt[:, h * 512:(h + 1) * 512],
                                    x1b[:, cs],
                                    k1p_sb[:, pr * 1024 + h * 512:
                                           pr * 1024 + (h + 1) * 512],
                                    start=True, stop=True)
                            pt_3d = pt[:].rearrange("p (m o) -> p m o", o=O)
                            vs_3d = v_3d[:, 8 * pr:8 * pr + 8]
                            eng = tbl[pr]
                            if eng == "D":
                                nc.vector.tensor_tensor(
                                    vs_3d, pt_3d, x0bc(pr), ALU.mult)
                            else:
                                nc.scalar.copy(v[:, ps], pt[:])
                                scale_eng = (nc.vector if eng == "A"
                                             else nc.gpsimd)
                                scale_eng.tensor_tensor(
                                    vs_3d, vs_3d, x0bc(pr), ALU.mult)

                        # ---- fold tree 40 -> 1 m-slices (fp16, SBUF)
                        nc.vector.tensor_tensor(
                            v[:, 0:2560], v[:, 0:2560], v[:, 2560:5120],
                            ALU.add)
                        nc.vector.tensor_tensor(
                            v[:, 0:1280], v[:, 0:1280], v[:, 1280:2560],
                            ALU.add)
                        nc.vector.tensor_tensor(
                            v[:, 0:640], v[:, 0:640], v[:, 640:1280],
                            ALU.add)
                        nc.vector.tensor_tensor(
                            v[:, 0:256], v[:, 0:256], v[:, 256:512],
                            ALU.add)
                        nc.vector.tensor_tensor(
                            v[:, 0:128], v[:, 0:128], v[:, 128:256],
                            ALU.add)
                        nc.vector.tensor_tensor(
                            v[:, 0:128], v[:, 0:128], v[:, 512:640],
                            ALU.add)

                        # ---- L2 indicator matmul for this chunk
                        pw = pw_pool.tile([128, M1 * 8], F32, name="pw",
                                          tag="pw")
                        nc.tensor.matmul(
                            pw[:], v[:, 0:128],
                            xe[:, (c - w * WIN) * M1 * 8:
                               (c - w * WIN + 1) * M1 * 8],
                            start=True, stop=True)
                        nc.scalar.copy(
                            w_4d[:, :, c * 8:(c + 1) * 8],
                            pw[:].rearrange("p (m e) -> p m e", e=8))

                    # ---- L2 tail for each completed 128-b block
                    if w % 4 == 3:
                        q = w // 4
                        po3 = po3_pool.tile([128, 128], F32, name="po3",
                                            tag="po3")
                        for m in range(M):
                            nc.tensor.matmul(
                                po3[:], k2p_sb[:, m * O:(m + 1) * O],
                                w_4d[:, m, q * 128:(q + 1) * 128],
                                start=(m == 0), stop=(m == M - 1))
                        nc.scalar.copy(o3_st[:, q * 128:(q + 1) * 128],
                                       po3[:])

            # ---- tail: out2, transpose + store
            with ExitStack() as tailst:
                ptp_pool = tailst.enter_context(
                    tc.tile_pool(name="ptpp", bufs=2, space="PSUM"))
                tb_pool = tailst.enter_context(
                    tc.tile_pool(name="tbs", bufs=3))

                nc.scalar.copy(o2_st[:], w_sb[:, M * bcl:M1 * bcl])
                for l, st in enumerate((o1_st, o2_st, o3_st)):
                    for j in range(nb):
                        tw = min(128, bcl - j * 128)
                        ptp = ptp_pool.tile([128, 128], F32, name="ptp",
                                            tag="ptp")
                        nc.tensor.transpose(
                            ptp[0:tw, :], st[:, j * 128:j * 128 + tw],
                            iden_sb[:])
                        tb = tb_pool.tile([128, 128], F32, name="tb",
                                          tag="tb")
                        nc.scalar.copy(tb[0:tw, :], ptp[0:tw, :])
                        nc.sync.dma_start(
                            out_d.ap()[j * 128:j * 128 + tw,
                                       l * O:(l + 1) * O],
                            tb[0:tw, :])

    _split_excess_waits(nc)
    return nc


_TRIU = np.triu_indices(M)


def host_prep(x0c, k0, k1, k2):
    """Per-core input prep. x0c: (bcl, M, D) float32."""
    bcl = x0c.shape[0]
    bd = bcl * D
    n_chunks = bd // 128
    x0m = np.ascontiguousarray(
        x0c.transpose(1, 0, 2).reshape(M, bd), dtype=np.float32)
    ia, ib = _TRIU
    # u-sym[(m<=m'), bd] = x0m[m]*x0m[m'], padded to 7*128 rows, fp16
    u = x0m[ia] * x0m[ib]
    u_pad = np.zeros((RB * 128, bd), np.float16)
    u_pad[0:NPAIR] = u.astype(np.float16)
    # k0s[(m<=m'), o] = k0[o,m,m'] + (m<m')*k0[o,m',m]
    k0s = k0[:, ia, ib] + np.where(ia == ib, 0.0, k0[:, ib, ia])
    k0s_pad = np.zeros((RB * 128, O), np.float16)
    k0s_pad[0:NPAIR] = k0s.T.astype(np.float16)

    x0t = np.concatenate(
        [x0c.transpose(0, 2, 1).reshape(bd, M),
         np.ones((bd, 1), np.float32)], axis=1)
    x0t = np.ascontiguousarray(x0t).astype(np.float16)

    k1p = np.ascontiguousarray(
        k1.transpose(1, 2, 0).reshape(O, M * O)).astype(np.float16)
    k2p = np.ascontiguousarray(
        k2.transpose(1, 2, 0).reshape(O, M * O)).astype(np.float16)

    e8 = (np.arange(128)[:, None] // D == np.arange(8)[None, :])
    e8 = e8.astype(np.float32)
    # x0e[p, (c, m, e)] = x0t[c*128+p, m] * e8[p, e]
    x0t_cm = x0t.astype(np.float32).reshape(n_chunks, 128, M1)
    x0e = (x0t_cm[:, :, :, None] * e8[None, :, None, :])
    x0e = np.ascontiguousarray(
        x0e.transpose(1, 0, 2, 3).reshape(128, n_chunks * M1 * 8)
    ).astype(np.float16)
    iden = np.eye(128, dtype=np.float32)
    return {"u": u_pad, "k0s": k0s_pad, "k1p": k1p, "k2p": k2p,
            "x0t": x0t, "x0e": x0e, "iden": iden}


_nc_cache = {}


def _get_nc(n_chunks):
    if n_chunks not in _nc_cache:
        _nc_cache[n_chunks] = build(n_chunks)
    return _nc_cache[n_chunks]


def kernel(x0, k0, k1, k2):
    from concourse.bass_utils import run_bass_kernel_spmd
    x0 = np.asarray(x0, dtype=np.float32)
    k0 = np.asarray(k0, dtype=np.float32)
    k1 = np.asarray(k1, dtype=np.float32)
    k2 = np.asarray(k2, dtype=np.float32)
    n_chunks = (BC * D) // 128
    nc = _get_nc(n_chunks)
    in_maps = [host_prep(x0[c * BC:(c + 1) * BC], k0, k1, k2)
               for c in range(N_CORES)]
    res = run_bass_kernel_spmd(nc, in_maps, core_ids=list(range(N_CORES)))
    out = np.concatenate([r["out"] for r in res.results], axis=0)
    return out.astype(np.float32)


# revision 14
# speedup vs baseline: 1.4059x; 1.4059x over previous
"""Trainium2 Bass kernel for nn_CIN: 3-layer compressed-interaction network.

Reference (per layer l, kernel k_l [O,H,M]):
    x_{l+1}[b,o,d] = sum_{h,m} x_l[b,h,d] * x0[b,m,d] * k_l[o,h,m]
    out = concat_l(sum_d x_{l+1}[b,o,d])          # (B, 3*128)

Sharding: pure data-parallel over B across 8 cores (512 batch each).

v4 design (fp16 end-to-end; rationale from cost-model study):
  * matmul cost = out-cols x 0.42ns, independent of contraction depth
    => deep PSUM accumulation is free, narrow outputs are cheap.
  * DVE/ACT elementwise ops on PSUM run at 1x (~0.55-0.66us per
    [128,512]); all-2-byte SBUF DVE ops can run 2-4x.

  L0: u-SYM route. u[(m<=m'),bd] = x0m[m]*x0m[m'] (820 rows, host,
  fp16) with k0s[(m,m'),o] = k0[o,m,m']+k0[o,m',m] folded: 7 row-blocks
  instead of 13 -> halves L0 PE time AND u DMA vs v3.
  y[o,bd-512] = 7 accumulating matmuls per window; ACT-drain to x1b.

  L1: per chunk, 10 matmuls pt_g[bd',4m*128o] in PSUM (lhsT = x1b
  chunk, shared). The m-contraction x2[bd,o] = sum_m pt[bd,(m,o)] *
  x0t[bd,m] is a mandatory 42M-elem PSUM drain pass; it is split
  across three engines by a tunable table:
    'D': DVE  tensor_tensor drain fused with the x0 broadcast-scale
    'A': ACT  copy drain (fp16), then DVE applies the scale (2x, SBUF)
    'G': GPSIMD tensor_tensor drain fused with the scale
  then a 6-op DVE fold-tree (all fp16 SBUF) sums the 40 m-slices.
  No diag DMA, no diag matmuls, no v_act bulk copies (v3 killed).

  L2: Gram-trick as v3: per chunk one indicator matmul
  pw[o,(m',e)] = v[:,0:128].T @ (x0t*E) gives G2 + out2; tail runs
  per 128-b block (40 accumulating [*,128] matmuls) for overlap.
"""

import numpy as np
from contextlib import ExitStack

import concourse.bass as bass
import concourse.tile as tile
import concourse.mybir as mybir

F32 = mybir.dt.float32
F16 = mybir.dt.float16
ALU = mybir.AluOpType

B, M, D, O = 4096, 40, 16, 128
N_CORES = 8
BC = B // N_CORES          # 512 batch rows per core
M1 = M + 1                 # x0t carries a trailing ones-column
NPAIR = M * (M + 1) // 2   # 820 symmetric (m<=m') pairs for L0
RB = 7                     # L0 row-blocks (896 rows incl pad)
WIN = 4                    # chunks per L0 window (512 cols)
NG = 10                    # pt groups per chunk (4 m's each)

# per-(c%2) engine for each of the 10 drain groups (1 PSUM bank each).
# GPSIMD cannot read PSUM, so drains are DVE or ACT only:
# P = ACT copy drain, PE diag-matmul applies scale AND fold (m 0..15)
# D = DVE tensor_tensor drain fused with the x0 scale, DVE fold tree
# G = ACT copy drain, GPSIMD applies the scale, DVE fold tree
DRAIN_TABLE = [
    "PPPPDDDDGG",
    "PPPPDDDGGG",
]
NPE = 16                   # m's on the PE diag path (groups 0..3)

_ns_ctr = [0]


def _split_excess_waits(nc, max_waits=1):
    """walrus in this env rejects >1 sync-wait on one instruction: move
    excess waits onto same-engine NoOps inserted before."""
    for f in nc.m.functions:
        for bb in f.blocks:
            new_list = []
            for inst in bb.instructions:
                si = inst.sync_info
                waits = list(si.on_wait) if si and si.on_wait else []
                if len(waits) > max_waits:
                    excess = waits[:-max_waits]
                    keep = waits[-max_waits:]
                    for i in range(0, len(excess), max_waits):
                        chunk = excess[i:i + max_waits]
                        _ns_ctr[0] += 1
                        nop = mybir.InstNoOp(
                            name=f"waitsplit-{_ns_ctr[0]}", ins=[], outs=[],
                            engine=inst.engine,
                            sync_info=mybir.SyncInfo(on_wait=chunk,
                                                     on_update=[]),
                        )
                        nc.register_instruction(nop)
                        new_list.append(nop)
                    si.on_wait = keep
                    inst.sync_info = si
                new_list.append(inst)
            bb.instructions[:] = new_list


def build(n_chunks):
    bd = n_chunks * 128
    bcl = bd // D              # local batch count
    nb = (bcl + 127) // 128    # output b-tiles
    nwin = n_chunks // WIN
    nc = bass.Bass("TRN2", target_bir_lowering=False, debug=False,
                   num_devices=1)

    u_d = nc.dram_tensor("u", [RB * 128, bd], F16, kind="ExternalInput")
    k0s_d = nc.dram_tensor("k0s", [RB * 128, O], F16, kind="ExternalInput")
    k1p_d = nc.dram_tensor("k1p", [O, M * O], F16, kind="ExternalInput")
    k2p_d = nc.dram_tensor("k2p", [O, M * O], F16, kind="ExternalInput")
    x0t_d = nc.dram_tensor("x0t", [bd, M1], F16, kind="ExternalInput")
    x0e_d = nc.dram_tensor("x0e", [128, n_chunks * M1 * 8], F16,
                           kind="ExternalInput")
    diag_d = nc.dram_tensor("diag", [128, n_chunks * NPE * 128], F16,
                            kind="ExternalInput")
    iden_d = nc.dram_tensor("iden", [128, 128], F32, kind="ExternalInput")
    out_d = nc.dram_tensor("out", [bcl, 3 * O], F32, kind="ExternalOutput")

    with tile.TileContext(nc) as tc:
        with ExitStack() as perm:
            pp = perm.enter_context(tc.tile_pool(name="perm", bufs=1))
            k0s_sb = pp.tile([128, RB * O], F16, name="k0s_sb")
            nc.sync.dma_start(
                k0s_sb[:].rearrange("p (j o) -> p j o", o=O),
                k0s_d.ap().rearrange("(j p) o -> p j o", p=128))
            k1p_sb = pp.tile([O, M * O], F16, name="k1p_sb")
            nc.sync.dma_start(k1p_sb[:], k1p_d.ap())
            k2p_sb = pp.tile([O, M * O], F16, name="k2p_sb")
            nc.sync.dma_start(k2p_sb[:], k2p_d.ap())
            x0t_sb = pp.tile([128, n_chunks * M1], F16, name="x0t_sb")
            nc.sync.dma_start(
                x0t_sb[:].rearrange("p (c m) -> p c m", m=M1),
                x0t_d.ap().rearrange("(c p) m -> p c m", p=128))
            iden_sb = pp.tile([128, 128], F32, name="iden_sb")
            nc.sync.dma_start(iden_sb[:], iden_d.ap())

            x1b = pp.tile([128, bd], F16, name="x1b")
            w_sb = pp.tile([128, M1 * bcl], F16, name="w_sb")
            o1_st = pp.tile([128, bcl], F32, name="o1_st")
            o2_st = pp.tile([128, bcl], F32, name="o2_st")
            o3_st = pp.tile([128, bcl], F32, name="o3_st")

            w_4d = w_sb[:].rearrange("p (m b) -> p m b", b=bcl)

            with ExitStack() as mainst:
                u_pool = mainst.enter_context(
                    tc.tile_pool(name="upool", bufs=2))
                v_pool = mainst.enter_context(
                    tc.tile_pool(name="vpool", bufs=2))
                vt_pool = mainst.enter_context(
                    tc.tile_pool(name="vtpool", bufs=2))
                x2_pool = mainst.enter_context(
                    tc.tile_pool(name="x2pool", bufs=2))
                dg_pool = mainst.enter_context(
                    tc.tile_pool(name="dgpool", bufs=2))
                xe_pool = mainst.enter_context(
                    tc.tile_pool(name="xepool", bufs=2))
                y_pool = mainst.enter_context(
                    tc.tile_pool(name="ypool", bufs=2, space="PSUM"))
                pt_pool = mainst.enter_context(
                    tc.tile_pool(name="ptpool", bufs=3, space="PSUM"))
                pw_pool = mainst.enter_context(
                    tc.tile_pool(name="pwpool", bufs=1, space="PSUM"))
                pf_pool = mainst.enter_context(
                    tc.tile_pool(name="pfpool", bufs=2, space="PSUM"))

                for w in range(nwin):
                    ws = slice(w * 512, (w + 1) * 512)
                    # ---- L0: 7 accumulating matmuls over host-built u-sym
                    u_sb = u_pool.tile([128, RB * 512], F16, name="u_sb",
                                       tag="u")
                    nc.sync.dma_start(
                        u_sb[:].rearrange("p (j c) -> p j c", c=512),
                        u_d.ap().rearrange("(j p) c -> p j c", p=128)
                        [:, :, ws])
                    xe = xe_pool.tile([128, WIN * M1 * 8], F16, name="xe",
                                      tag="xe")
                    nc.sync.dma_start(
                        xe[:], x0e_d.ap()[:, w * WIN * M1 * 8:
                                          (w + 1) * WIN * M1 * 8])
                    y = y_pool.tile([128, 512], F32, name="y", tag="y")
                    for j in range(RB):
                        nc.tensor.matmul(
                            y[:], k0s_sb[:, j * O:(j + 1) * O],
                            u_sb[:, j * 512:(j + 1) * 512],
                            start=(j == 0), stop=(j == RB - 1))
                    nc.scalar.copy(x1b[:, ws], y[:])

                    if w % 4 == 3:
                        q = w // 4
                        nc.vector.tensor_reduce(
                            o1_st[:, q * 128:(q + 1) * 128],
                            x1b[:, q * 2048:(q + 1) * 2048]
                            .rearrange("p (b d) -> p b d", d=D),
                            mybir.AxisListType.X, ALU.add)

                    for c in range(w * WIN, (w + 1) * WIN):
                        cs = slice(c * 128, (c + 1) * 128)
                        tbl = DRAIN_TABLE[c % len(DRAIN_TABLE)]

                        def x0bc(g):
                            return (x0t_sb[:, c * M1 + 4 * g:
                                           c * M1 + 4 * g + 4]
                                    .unsqueeze(2).broadcast_to([128, 4, 128]))

                        # ---- L1: 10 matmul groups, 3-path scaled drain
                        v = v_pool.tile([128, NG * 512], F16, name="v",
                                        tag="v")
                        v_3d = v[:].rearrange("p (m o) -> p m o", o=O)
                        dg = dg_pool.tile([128, NPE * 128], F16, name="dg",
                                          tag="dg")
                        nc.sync.dma_start(
                            dg[:], diag_d.ap()[:, c * NPE * 128:
                                               (c + 1) * NPE * 128])
                        pfold = pf_pool.tile([128, 128], F32, name="pfold",
                                             tag="pf")
                        for g in range(NG):
                            pt = pt_pool.tile([128, 512], F32, name="pt",
                                              tag="pt")
                            gs = slice(g * 512, (g + 1) * 512)
                            nc.tensor.matmul(pt[:], x1b[:, cs],
                                             k1p_sb[:, gs],
                                             start=True, stop=True)
                            eng = tbl[g]
                            if eng == "D":
                                nc.vector.tensor_tensor(
                                    v_3d[:, 4 * g:4 * g + 4],
                                    pt[:].rearrange("p (m o) -> p m o", o=O),
                                    x0bc(g), ALU.mult)
                            elif eng == "G":
                                nc.scalar.copy(v[:, gs], pt[:])
                                nc.gpsimd.tensor_tensor(
                                    v_3d[:, 4 * g:4 * g + 4],
                                    v_3d[:, 4 * g:4 * g + 4],
                                    x0bc(g), ALU.mult)
                            else:  # P: raw copy, PE diag fold+scale
                                nc.scalar.copy(v[:, gs], pt[:])
                                for i in range(4 * g, 4 * g + 4):
                                    nc.tensor.matmul(
                                        pfold[:],
                                        dg[:, i * 128:(i + 1) * 128],
                                        v[:, i * 128:(i + 1) * 128],
                                        start=(i == 0), stop=(i == NPE - 1))

                        # ---- fold tree over m 16..39 (fp16 SBUF, no alias)
                        vt = vt_pool.tile([128, 2944], F16, name="vt",
                                          tag="vt")
                        nc.vector.tensor_tensor(
                            vt[:, 0:1536], v[:, 2048:3584], v[:, 3584:5120],
                            ALU.add)
                        nc.vector.tensor_tensor(
                            vt[:, 1536:2304], vt[:, 0:768], vt[:, 768:1536],
                            ALU.add)
                        nc.vector.tensor_tensor(
                            vt[:, 2304:2688], vt[:, 1536:1920],
                            vt[:, 1920:2304], ALU.add)
                        nc.vector.tensor_tensor(
                            vt[:, 2688:2816], vt[:, 2304:2432],
                            vt[:, 2432:2560], ALU.add)
                        nc.vector.tensor_tensor(
                            vt[:, 2816:2944], vt[:, 2688:2816],
                            vt[:, 2560:2688], ALU.add)
                        # merge with the PE pfold (PSUM f32)
                        x2t = x2_pool.tile([128, 128], F16, name="x2t",
                                           tag="x2")
                        nc.vector.tensor_tensor(
                            x2t[:], vt[:, 2816:2944], pfold[:], ALU.add)

                        # ---- L2 indicator matmul for this chunk
                        pw = pw_pool.tile([128, M1 * 8], F32, name="pw",
                                          tag="pw")
                        nc.tensor.matmul(
                            pw[:], x2t[:],
                            xe[:, (c - w * WIN) * M1 * 8:
                               (c - w * WIN + 1) * M1 * 8],
                            start=True, stop=True)
                        nc.scalar.copy(
                            w_4d[:, :, c * 8:(c + 1) * 8],
                            pw[:].rearrange("p (m e) -> p m e", e=8))

                    # ---- L2 tail for each completed 128-b block
                    # (po3 borrows the pw pool's PSUM bank)
                    if w % 4 == 3:
                        q = w // 4
                        po3 = pw_pool.tile([128, M1 * 8], F32, name="po3",
                                           tag="pw")
                        for m in range(M):
                            nc.tensor.matmul(
                                po3[:, 0:128], k2p_sb[:, m * O:(m + 1) * O],
                                w_4d[:, m, q * 128:(q + 1) * 128],
                                start=(m == 0), stop=(m == M - 1))
                        nc.scalar.copy(o3_st[:, q * 128:(q + 1) * 128],
                                       po3[:, 0:128])

            # ---- tail: out2, transpose + store
            with ExitStack() as tailst:
                ptp_pool = tailst.enter_context(
                    tc.tile_pool(name="ptpp", bufs=2, space="PSUM"))
                tb_pool = tailst.enter_context(
                    tc.tile_pool(name="tbs", bufs=3))

                nc.scalar.copy(o2_st[:], w_sb[:, M * bcl:M1 * bcl])
                for l, st in enumerate((o1_st, o2_st, o3_st)):
                    for j in range(nb):
                        tw = min(128, bcl - j * 128)
                        ptp = ptp_pool.tile([128, 128], F32, name="ptp",
                                            tag="ptp")
                        nc.tensor.transpose(
                            ptp[0:tw, :], st[:, j * 128:j * 128 + tw],
                            iden_sb[:])
                        tb = tb_pool.tile([128, 128], F32, name="tb",
                                          tag="tb")
                        nc.scalar.copy(tb[0:tw, :], ptp[0:tw, :])
                        nc.sync.dma_start(
                            out_d.ap()[j * 128:j * 128 + tw,
                                       l * O:(l + 1) * O],
                            tb[0:tw, :])

    _split_excess_waits(nc)
    return nc


_TRIU = np.triu_indices(M)


def host_prep(x0c, k0, k1, k2):
    """Per-core input prep. x0c: (bcl, M, D) float32."""
    bcl = x0c.shape[0]
    bd = bcl * D
    n_chunks = bd // 128
    x0m = np.ascontiguousarray(
        x0c.transpose(1, 0, 2).reshape(M, bd), dtype=np.float32)
    ia, ib = _TRIU
    # u-sym[(m<=m'), bd] = x0m[m]*x0m[m'], padded to 7*128 rows, fp16
    u = x0m[ia] * x0m[ib]
    u_pad = np.zeros((RB * 128, bd), np.float16)
    u_pad[0:NPAIR] = u.astype(np.float16)
    # k0s[(m<=m'), o] = k0[o,m,m'] + (m<m')*k0[o,m',m]
    k0s = k0[:, ia, ib] + np.where(ia == ib, 0.0, k0[:, ib, ia])
    k0s_pad = np.zeros((RB * 128, O), np.float16)
    k0s_pad[0:NPAIR] = k0s.T.astype(np.float16)

    x0t = np.concatenate(
        [x0c.transpose(0, 2, 1).reshape(bd, M),
         np.ones((bd, 1), np.float32)], axis=1)
    x0t = np.ascontiguousarray(x0t).astype(np.float16)

    k1p = np.ascontiguousarray(
        k1.transpose(1, 2, 0).reshape(O, M * O)).astype(np.float16)
    k2p = np.ascontiguousarray(
        k2.transpose(1, 2, 0).reshape(O, M * O)).astype(np.float16)

    # diag tiles for the PE fold path: m = 0..NPE-1
    # diag[c][p, i*128+q] = x0t[c*128+p, i] * (p==q)
    x0t32 = x0t.astype(np.float32)
    dd = np.zeros((n_chunks, 128, NPE, 128), np.float32)
    x0t_c = x0t32[:, 0:NPE].reshape(n_chunks, 128, NPE)
    idx = np.arange(128)
    dd[:, idx, :, idx] = x0t_c.transpose(1, 0, 2)
    diag = np.ascontiguousarray(
        dd.transpose(1, 0, 2, 3).reshape(128, n_chunks * NPE * 128)
    ).astype(np.float16)

    e8 = (np.arange(128)[:, None] // D == np.arange(8)[None, :])
    e8 = e8.astype(np.float32)
    # x0e[p, (c, m, e)] = x0t[c*128+p, m] * e8[p, e]
    x0t_cm = x0t.astype(np.float32).reshape(n_chunks, 128, M1)
    x0e = (x0t_cm[:, :, :, None] * e8[None, :, None, :])
    x0e = np.ascontiguousarray(
        x0e.transpose(1, 0, 2, 3).reshape(128, n_chunks * M1 * 8)
    ).astype(np.float16)
    iden = np.eye(128, dtype=np.float32)
    return {"u": u_pad, "k0s": k0s_pad, "k1p": k1p, "k2p": k2p,
            "x0t": x0t, "x0e": x0e, "diag": diag, "iden": iden}


_nc_cache = {}


def _get_nc(n_chunks):
    if n_chunks not in _nc_cache:
        _nc_cache[n_chunks] = build(n_chunks)
    return _nc_cache[n_chunks]


def kernel(x0, k0, k1, k2):
    from concourse.bass_utils import run_bass_kernel_spmd
    x0 = np.asarray(x0, dtype=np.float32)
    k0 = np.asarray(k0, dtype=np.float32)
    k1 = np.asarray(k1, dtype=np.float32)
    k2 = np.asarray(k2, dtype=np.float32)
    n_chunks = (BC * D) // 128
    nc = _get_nc(n_chunks)
    in_maps = [host_prep(x0[c * BC:(c + 1) * BC], k0, k1, k2)
               for c in range(N_CORES)]
    res = run_bass_kernel_spmd(nc, in_maps, core_ids=list(range(N_CORES)))
    out = np.concatenate([r["out"] for r in res.results], axis=0)
    return out.astype(np.float32)


# revision 15
# speedup vs baseline: 1.5854x; 1.1276x over previous
"""Trainium2 Bass kernel for nn_CIN: 3-layer compressed-interaction network.

Reference (per layer l, kernel k_l [O,H,M]):
    x_{l+1}[b,o,d] = sum_{h,m} x_l[b,h,d] * x0[b,m,d] * k_l[o,h,m]
    out = concat_l(sum_d x_{l+1}[b,o,d])          # (B, 3*128)

Sharding: pure data-parallel over B across 8 cores (512 batch each).

v4 design (fp16 end-to-end; rationale from cost-model study):
  * matmul cost = out-cols x 0.42ns, independent of contraction depth
    => deep PSUM accumulation is free, narrow outputs are cheap.
  * DVE/ACT elementwise ops on PSUM run at 1x (~0.55-0.66us per
    [128,512]); all-2-byte SBUF DVE ops can run 2-4x.

  L0: u-SYM route. u[(m<=m'),bd] = x0m[m]*x0m[m'] (820 rows, host,
  fp16) with k0s[(m,m'),o] = k0[o,m,m']+k0[o,m',m] folded: 7 row-blocks
  instead of 13 -> halves L0 PE time AND u DMA vs v3.
  y[o,bd-512] = 7 accumulating matmuls per window; ACT-drain to x1b.

  L1: per chunk, 10 matmuls pt_g[bd',4m*128o] in PSUM (lhsT = x1b
  chunk, shared). The m-contraction x2[bd,o] = sum_m pt[bd,(m,o)] *
  x0t[bd,m] is a mandatory 42M-elem PSUM drain pass; it is split
  across three engines by a tunable table:
    'D': DVE  tensor_tensor drain fused with the x0 broadcast-scale
    'A': ACT  copy drain (fp16), then DVE applies the scale (2x, SBUF)
    'G': GPSIMD tensor_tensor drain fused with the scale
  then a 6-op DVE fold-tree (all fp16 SBUF) sums the 40 m-slices.
  No diag DMA, no diag matmuls, no v_act bulk copies (v3 killed).

  L2: Gram-trick as v3: per chunk one indicator matmul
  pw[o,(m',e)] = v[:,0:128].T @ (x0t*E) gives G2 + out2; tail runs
  per 128-b block (40 accumulating [*,128] matmuls) for overlap.
"""

import numpy as np
from contextlib import ExitStack

import concourse.bass as bass
import concourse.tile as tile
import concourse.mybir as mybir

F32 = mybir.dt.float32
F16 = mybir.dt.float16
ALU = mybir.AluOpType

B, M, D, O = 4096, 40, 16, 128
N_CORES = 8
BC = B // N_CORES          # 512 batch rows per core
M1 = M + 1                 # x0t carries a trailing ones-column
NPAIR = M * (M + 1) // 2   # 820 symmetric (m<=m') pairs for L0
RB = 7                     # L0 row-blocks (896 rows incl pad)
WIN = 4                    # chunks per L0 window (512 cols)
NG = 10                    # pt groups per chunk (4 m's each)

# per-(c%2) engine for each of the 10 drain groups (1 PSUM bank each).
# GPSIMD cannot read PSUM, so drains are DVE or ACT only:
# P = ACT copy drain, PE diag-matmul applies scale AND fold (m 0..15)
# D = DVE tensor_tensor drain fused with the x0 scale, DVE fold tree
# G = ACT copy drain, GPSIMD applies the scale, DVE fold tree
DRAIN_TABLE = [
    "PPPPGGDDDD",
    "PPPPGGGDDD",
]
NPE = 16                   # m's on the PE diag path (groups 0..3)

_ns_ctr = [0]


def _split_excess_waits(nc, max_waits=1):
    """walrus in this env rejects >1 sync-wait on one instruction: move
    excess waits onto same-engine NoOps inserted before."""
    for f in nc.m.functions:
        for bb in f.blocks:
            new_list = []
            for inst in bb.instructions:
                si = inst.sync_info
                waits = list(si.on_wait) if si and si.on_wait else []
                if len(waits) > max_waits:
                    excess = waits[:-max_waits]
                    keep = waits[-max_waits:]
                    for i in range(0, len(excess), max_waits):
                        chunk = excess[i:i + max_waits]
                        _ns_ctr[0] += 1
                        nop = mybir.InstNoOp(
                            name=f"waitsplit-{_ns_ctr[0]}", ins=[], outs=[],
                            engine=inst.engine,
                            sync_info=mybir.SyncInfo(on_wait=chunk,
                                                     on_update=[]),
                        )
                        nc.register_instruction(nop)
                        new_list.append(nop)
                    si.on_wait = keep
                    inst.sync_info = si
                new_list.append(inst)
            bb.instructions[:] = new_list


def build(n_chunks):
    bd = n_chunks * 128
    bcl = bd // D              # local batch count
    nb = (bcl + 127) // 128    # output b-tiles
    nwin = n_chunks // WIN
    nc = bass.Bass("TRN2", target_bir_lowering=False, debug=False,
                   num_devices=1)

    u_d = nc.dram_tensor("u", [RB * 128, bd], F16, kind="ExternalInput")
    k0s_d = nc.dram_tensor("k0s", [RB * 128, O], F16, kind="ExternalInput")
    k1p_d = nc.dram_tensor("k1p", [O, M * O], F16, kind="ExternalInput")
    k2p_d = nc.dram_tensor("k2p", [O, M * O], F16, kind="ExternalInput")
    x0t_d = nc.dram_tensor("x0t", [bd, M1], F16, kind="ExternalInput")
    x0e_d = nc.dram_tensor("x0e", [128, n_chunks * M1 * 8], F16,
                           kind="ExternalInput")
    diag_d = nc.dram_tensor("diag", [128, n_chunks * NPE * 128], F16,
                            kind="ExternalInput")
    iden_d = nc.dram_tensor("iden", [128, 128], F32, kind="ExternalInput")
    out_d = nc.dram_tensor("out", [bcl, 3 * O], F32, kind="ExternalOutput")

    with tile.TileContext(nc) as tc:
        with ExitStack() as perm:
            pp = perm.enter_context(tc.tile_pool(name="perm", bufs=1))
            k0s_sb = pp.tile([128, RB * O], F16, name="k0s_sb")
            nc.sync.dma_start(
                k0s_sb[:].rearrange("p (j o) -> p j o", o=O),
                k0s_d.ap().rearrange("(j p) o -> p j o", p=128))
            k1p_sb = pp.tile([O, M * O], F16, name="k1p_sb")
            nc.sync.dma_start(k1p_sb[:], k1p_d.ap())
            k2p_sb = pp.tile([O, M * O], F16, name="k2p_sb")
            nc.sync.dma_start(k2p_sb[:], k2p_d.ap())
            x0t_sb = pp.tile([128, n_chunks * M1], F16, name="x0t_sb")
            nc.sync.dma_start(
                x0t_sb[:].rearrange("p (c m) -> p c m", m=M1),
                x0t_d.ap().rearrange("(c p) m -> p c m", p=128))
            iden_sb = pp.tile([128, 128], F32, name="iden_sb")
            nc.sync.dma_start(iden_sb[:], iden_d.ap())

            x1b = pp.tile([128, bd], F16, name="x1b")
            w_sb = pp.tile([128, M1 * bcl], F16, name="w_sb")
            o1_st = pp.tile([128, bcl], F32, name="o1_st")
            o2_st = pp.tile([128, bcl], F32, name="o2_st")
            o3_st = pp.tile([128, bcl], F32, name="o3_st")

            w_4d = w_sb[:].rearrange("p (m b) -> p m b", b=bcl)

            with ExitStack() as mainst:
                u_pool = mainst.enter_context(
                    tc.tile_pool(name="upool", bufs=2))
                v_pool = mainst.enter_context(
                    tc.tile_pool(name="vpool", bufs=3))
                vt_pool = mainst.enter_context(
                    tc.tile_pool(name="vtpool", bufs=3))
                x2_pool = mainst.enter_context(
                    tc.tile_pool(name="x2pool", bufs=3))
                dg_pool = mainst.enter_context(
                    tc.tile_pool(name="dgpool", bufs=3))
                xe_pool = mainst.enter_context(
                    tc.tile_pool(name="xepool", bufs=2))
                y_pool = mainst.enter_context(
                    tc.tile_pool(name="ypool", bufs=1, space="PSUM"))
                pt_pool = mainst.enter_context(
                    tc.tile_pool(name="ptpool", bufs=3, space="PSUM"))
                pw_pool = mainst.enter_context(
                    tc.tile_pool(name="pwpool", bufs=2, space="PSUM"))
                pf_pool = mainst.enter_context(
                    tc.tile_pool(name="pfpool", bufs=2, space="PSUM"))

                for w in range(nwin):
                    ws = slice(w * 512, (w + 1) * 512)
                    # ---- L0: 7 accumulating matmuls over host-built u-sym
                    u_sb = u_pool.tile([128, RB * 512], F16, name="u_sb",
                                       tag="u")
                    nc.sync.dma_start(
                        u_sb[:].rearrange("p (j c) -> p j c", c=512),
                        u_d.ap().rearrange("(j p) c -> p j c", p=128)
                        [:, :, ws])
                    xe = xe_pool.tile([128, WIN * M1 * 8], F16, name="xe",
                                      tag="xe")
                    nc.sync.dma_start(
                        xe[:], x0e_d.ap()[:, w * WIN * M1 * 8:
                                          (w + 1) * WIN * M1 * 8])
                    y = y_pool.tile([128, 512], F32, name="y", tag="y")
                    for j in range(RB):
                        nc.tensor.matmul(
                            y[:], k0s_sb[:, j * O:(j + 1) * O],
                            u_sb[:, j * 512:(j + 1) * 512],
                            start=(j == 0), stop=(j == RB - 1))
                    nc.scalar.copy(x1b[:, ws], y[:])

                    if w % 4 == 3:
                        q = w // 4
                        nc.vector.tensor_reduce(
                            o1_st[:, q * 128:(q + 1) * 128],
                            x1b[:, q * 2048:(q + 1) * 2048]
                            .rearrange("p (b d) -> p b d", d=D),
                            mybir.AxisListType.X, ALU.add)

                    for c in range(w * WIN, (w + 1) * WIN):
                        cs = slice(c * 128, (c + 1) * 128)
                        tbl = DRAIN_TABLE[c % len(DRAIN_TABLE)]

                        def x0bc(g):
                            return (x0t_sb[:, c * M1 + 4 * g:
                                           c * M1 + 4 * g + 4]
                                    .unsqueeze(2).broadcast_to([128, 4, 128]))

                        # ---- L1: 10 matmul groups, 3-path scaled drain
                        v = v_pool.tile([128, NG * 512], F16, name="v",
                                        tag="v")
                        v_3d = v[:].rearrange("p (m o) -> p m o", o=O)
                        dg = dg_pool.tile([128, NPE * 128], F16, name="dg",
                                          tag="dg")
                        nc.sync.dma_start(
                            dg[:], diag_d.ap()[:, c * NPE * 128:
                                               (c + 1) * NPE * 128])
                        pfold = pf_pool.tile([128, 128], F32, name="pfold",
                                             tag="pf")
                        for g in range(NG):
                            pt = pt_pool.tile([128, 512], F32, name="pt",
                                              tag="pt")
                            gs = slice(g * 512, (g + 1) * 512)
                            nc.tensor.matmul(pt[:], x1b[:, cs],
                                             k1p_sb[:, gs],
                                             start=True, stop=True)
                            eng = tbl[g]
                            if eng == "D":
                                nc.vector.tensor_tensor(
                                    v_3d[:, 4 * g:4 * g + 4],
                                    pt[:].rearrange("p (m o) -> p m o", o=O),
                                    x0bc(g), ALU.mult)
                            elif eng == "G":
                                nc.scalar.copy(v[:, gs], pt[:])
                                nc.gpsimd.tensor_tensor(
                                    v_3d[:, 4 * g:4 * g + 4],
                                    v_3d[:, 4 * g:4 * g + 4],
                                    x0bc(g), ALU.mult)
                            else:  # P: raw copy, PE diag fold+scale
                                nc.scalar.copy(v[:, gs], pt[:])
                                for i in range(4 * g, 4 * g + 4):
                                    nc.tensor.matmul(
                                        pfold[:],
                                        dg[:, i * 128:(i + 1) * 128],
                                        v[:, i * 128:(i + 1) * 128],
                                        start=(i == 0), stop=(i == NPE - 1))

                        # ---- fold tree over m 16..39 (fp16 SBUF, no alias)
                        vt = vt_pool.tile([128, 2944], F16, name="vt",
                                          tag="vt")
                        nc.vector.tensor_tensor(
                            vt[:, 0:1536], v[:, 2048:3584], v[:, 3584:5120],
                            ALU.add)
                        nc.vector.tensor_tensor(
                            vt[:, 1536:2304], vt[:, 0:768], vt[:, 768:1536],
                            ALU.add)
                        nc.vector.tensor_tensor(
                            vt[:, 2304:2688], vt[:, 1536:1920],
                            vt[:, 1920:2304], ALU.add)
                        nc.vector.tensor_tensor(
                            vt[:, 2688:2816], vt[:, 2304:2432],
                            vt[:, 2432:2560], ALU.add)
                        nc.vector.tensor_tensor(
                            vt[:, 2816:2944], vt[:, 2688:2816],
                            vt[:, 2560:2688], ALU.add)
                        # merge with the PE pfold (PSUM f32)
                        x2t = x2_pool.tile([128, 128], F16, name="x2t",
                                           tag="x2")
                        nc.vector.tensor_tensor(
                            x2t[:], vt[:, 2816:2944], pfold[:], ALU.add)

                        # ---- L2 indicator matmul for this chunk
                        pw = pw_pool.tile([128, M1 * 8], F32, name="pw",
                                          tag="pw")
                        nc.tensor.matmul(
                            pw[:], x2t[:],
                            xe[:, (c - w * WIN) * M1 * 8:
                               (c - w * WIN + 1) * M1 * 8],
                            start=True, stop=True)
                        nc.scalar.copy(
                            w_4d[:, :, c * 8:(c + 1) * 8],
                            pw[:].rearrange("p (m e) -> p m e", e=8))

                    # ---- L2 tail for each completed 128-b block
                    # (po3 borrows the pw pool's PSUM bank)
                    if w % 4 == 3:
                        q = w // 4
                        po3 = pw_pool.tile([128, M1 * 8], F32, name="po3",
                                           tag="pw")
                        for m in range(M):
                            nc.tensor.matmul(
                                po3[:, 0:128], k2p_sb[:, m * O:(m + 1) * O],
                                w_4d[:, m, q * 128:(q + 1) * 128],
                                start=(m == 0), stop=(m == M - 1))
                        nc.scalar.copy(o3_st[:, q * 128:(q + 1) * 128],
                                       po3[:, 0:128])

            # ---- tail: out2, transpose + store
            with ExitStack() as tailst:
                ptp_pool = tailst.enter_context(
                    tc.tile_pool(name="ptpp", bufs=2, space="PSUM"))
                tb_pool = tailst.enter_context(
                    tc.tile_pool(name="tbs", bufs=3))

                nc.scalar.copy(o2_st[:], w_sb[:, M * bcl:M1 * bcl])
                for l, st in enumerate((o1_st, o2_st, o3_st)):
                    for j in range(nb):
                        tw = min(128, bcl - j * 128)
                        ptp = ptp_pool.tile([128, 128], F32, name="ptp",
                                            tag="ptp")
                        nc.tensor.transpose(
                            ptp[0:tw, :], st[:, j * 128:j * 128 + tw],
                            iden_sb[:])
                        tb = tb_pool.tile([128, 128], F32, name="tb",
                                          tag="tb")
                        nc.scalar.copy(tb[0:tw, :], ptp[0:tw, :])
                        nc.sync.dma_start(
                            out_d.ap()[j * 128:j * 128 + tw,
                                       l * O:(l + 1) * O],
                            tb[0:tw, :])

    _split_excess_waits(nc)
    return nc


_TRIU = np.triu_indices(M)


def host_prep(x0c, k0, k1, k2):
    """Per-core input prep. x0c: (bcl, M, D) float32."""
    bcl = x0c.shape[0]
    bd = bcl * D
    n_chunks = bd // 128
    x0m = np.ascontiguousarray(
        x0c.transpose(1, 0, 2).reshape(M, bd), dtype=np.float32)
    ia, ib = _TRIU
    # u-sym[(m<=m'), bd] = x0m[m]*x0m[m'], padded to 7*128 rows, fp16
    u = x0m[ia] * x0m[ib]
    u_pad = np.zeros((RB * 128, bd), np.float16)
    u_pad[0:NPAIR] = u.astype(np.float16)
    # k0s[(m<=m'), o] = k0[o,m,m'] + (m<m')*k0[o,m',m]
    k0s = k0[:, ia, ib] + np.where(ia == ib, 0.0, k0[:, ib, ia])
    k0s_pad = np.zeros((RB * 128, O), np.float16)
    k0s_pad[0:NPAIR] = k0s.T.astype(np.float16)

    x0t = np.concatenate(
        [x0c.transpose(0, 2, 1).reshape(bd, M),
         np.ones((bd, 1), np.float32)], axis=1)
    x0t = np.ascontiguousarray(x0t).astype(np.float16)

    k1p = np.ascontiguousarray(
        k1.transpose(1, 2, 0).reshape(O, M * O)).astype(np.float16)
    k2p = np.ascontiguousarray(
        k2.transpose(1, 2, 0).reshape(O, M * O)).astype(np.float16)

    # diag tiles for the PE fold path: m = 0..NPE-1
    # diag[c][p, i*128+q] = x0t[c*128+p, i] * (p==q)
    x0t32 = x0t.astype(np.float32)
    dd = np.zeros((n_chunks, 128, NPE, 128), np.float32)
    x0t_c = x0t32[:, 0:NPE].reshape(n_chunks, 128, NPE)
    idx = np.arange(128)
    dd[:, idx, :, idx] = x0t_c.transpose(1, 0, 2)
    diag = np.ascontiguousarray(
        dd.transpose(1, 0, 2, 3).reshape(128, n_chunks * NPE * 128)
    ).astype(np.float16)

    e8 = (np.arange(128)[:, None] // D == np.arange(8)[None, :])
    e8 = e8.astype(np.float32)
    # x0e[p, (c, m, e)] = x0t[c*128+p, m] * e8[p, e]
    x0t_cm = x0t.astype(np.float32).reshape(n_chunks, 128, M1)
    x0e = (x0t_cm[:, :, :, None] * e8[None, :, None, :])
    x0e = np.ascontiguousarray(
        x0e.transpose(1, 0, 2, 3).reshape(128, n_chunks * M1 * 8)
    ).astype(np.float16)
    iden = np.eye(128, dtype=np.float32)
    return {"u": u_pad, "k0s": k0s_pad, "k1p": k1p, "k2p": k2p,
            "x0t": x0t, "x0e": x0e, "diag": diag, "iden": iden}


_nc_cache = {}


def _get_nc(n_chunks):
    if n_chunks not in _nc_cache:
        _nc_cache[n_chunks] = build(n_chunks)
    return _nc_cache[n_chunks]


def kernel(x0, k0, k1, k2):
    from concourse.bass_utils import run_bass_kernel_spmd
    x0 = np.asarray(x0, dtype=np.float32)
    k0 = np.asarray(k0, dtype=np.float32)
    k1 = np.asarray(k1, dtype=np.float32)
    k2 = np.asarray(k2, dtype=np.float32)
    n_chunks = (BC * D) // 128
    nc = _get_nc(n_chunks)
    in_maps = [host_prep(x0[c * BC:(c + 1) * BC], k0, k1, k2)
               for c in range(N_CORES)]
    res = run_bass_kernel_spmd(nc, in_maps, core_ids=list(range(N_CORES)))
    out = np.concatenate([r["out"] for r in res.results], axis=0)
    return out.astype(np.float32)
